# revision 6
# baseline (speedup 1.0000x reference)
"""Trainium2 Bass kernel for AlignOnlySubLayer.

Per batch b:
    W[c,m]   = sum_d context[b,c,d] * main[b,m,d]
    A        = softmax(W, axis=m)
    out[m,d] = main[b,m,d] - sum_c A[c,m] * context[b,c,d]

Sharding: data-parallel over batch B=8 across the 8 NeuronCores (one batch
per core, no cross-core communication).

Kernel strategy (per core):
  - Both matmuls contract with d (resp. c) on the partition axis, so mm1
    needs d-major (transposed) copies of context/main, built with the DMA
    xbar transpose (2-byte dtypes only).
  - Matmuls run in 16-bit: on trn2 the PE runs fp32 matmuls at 1/4 rate
    (2 half-speed passes) while fp16/bf16 stream 1 row/cycle. mm1 uses
    fp16 (4x more mantissa than bf16). mm2's operands (E, ctx_s) must be
    bf16: E = exp(W) reaches ~e^70 without max-subtraction, far beyond
    fp16 range, while bf16 keeps the f32 exponent. End-to-end error is
    ~2e-3 relative l2 (~3.5e-3 scale-relative absmax).
  - mm1 (fp16): W tiles [c=128, m=2048] into rotating PSUM half-buffers.
  - Softmax without max-subtraction (|W| <~ 70 << 88.7 fp32 exp overflow
    limit; softmax is shift-invariant so results match the reference):
    ACT Exp reads PSUM directly, writes fp16 E, and the fused accum_out
    gives the row sums S[c] for free - no separate reduce pass.
  - Normalization folded into context: ctx_s[c,:] = context[c,:] / S[c]
    (scales a 2048x128 matrix instead of the 2048x2048 weights).
  - mm2 (fp16): weightedT[d,m] += ctx_s[c-chunk].T @ E[c-chunk] accumulated
    over all 16 c-chunks in a persistent 4-bank PSUM region, N=512 matmuls.
  - Tail: weightedT -> fp16 hi/lo pair, xbar-transpose back to [m,d], and
    out = ((main - hi) - lo) in f32, streamed per quarter.

PRECISE=True switches the matmuls to f32 (inputs reconstructed from fp16
hi/lo transposed pairs): ~4e-6 relative error but ~4x slower matmuls.
"""

import numpy as np

import concourse.bass as bass
import concourse.mybir as mybir
from concourse import bacc
from concourse.tile import TileContext
from concourse.bass_utils import run_bass_kernel_spmd

P = 128
F32 = mybir.dt.float32
F16 = mybir.dt.float16
BF16 = mybir.dt.bfloat16
EXP = mybir.ActivationFunctionType.Exp
N_CORES = 8
PRECISE = False
TAIL_HILO = True


def build_nc(S=2048, D=128, num_devices=N_CORES, repeats=1, precise=PRECISE):
    """Build the single-core Bass program (SPMD across cores)."""
    assert D == P and S % P == 0
    T = S // P            # number of 128-row tiles along c (and m)
    QT = max(1, T // 4)   # tiles per prologue/tail chunk
    NQ = T // QT          # number of chunks
    QW = QT * P           # columns per chunk
    HALF = S // 2         # columns per mm1 psum half
    MMDT = F32 if precise else F16   # mm1 operand dtype
    EDT = F32 if precise else BF16   # mm2 operand dtype (E overflows fp16)

    nc = bacc.Bacc(
        "TRN2",
        target_bir_lowering=False,
        debug=False,
        enable_asserts=False,
        num_devices=num_devices,
    )
    ctx_d = nc.dram_tensor("context", [S, D], F32, kind="ExternalInput").ap()
    main_d = nc.dram_tensor("main", [S, D], F32, kind="ExternalInput").ap()
    out_d = nc.dram_tensor("out", [S, D], F32, kind="ExternalOutput").ap()

    ctx_dt = ctx_d.rearrange("(t p) d -> p t d", p=P)
    main_dt = main_d.rearrange("(t p) d -> p t d", p=P)
    out_dt = out_d.rearrange("(t p) d -> p t d", p=P)

    with TileContext(nc) as tc:
      for _rep in range(repeats):
        with (
            tc.tile_pool(name="persist", bufs=1) as persist,
            tc.tile_pool(name="prolog", bufs=2) as prolog,
            tc.tile_pool(name="etile", bufs=3) as etile_pool,
            tc.tile_pool(name="small", bufs=3) as small,
            tc.tile_pool(name="tailp", bufs=2) as tailp,
            tc.tile_pool(name="psum_w", bufs=2, space="PSUM") as psum_w,
            tc.tile_pool(name="psum_acc", bufs=1, space="PSUM") as psum_acc,
        ):
            # ---- persistent SBUF tensors ----
            ctx_f32 = persist.tile([P, T, P], F32)     # natural [c_in, ct, d]
            main_f32 = persist.tile([P, T, P], F32)    # natural [m_in, mt, d]
            ctxT = persist.tile([P, T, P], MMDT)       # [d, ct, c_in]
            mainT = persist.tile([P, T, P], MMDT)      # [d, mt, m_in]
            mainT2 = mainT.rearrange("p a b -> p (a b)")

            # Warm the ACT exp table early so the ~2.7us table load overlaps
            # the prologue DMAs.
            warm = small.tile([P, 1], F32, tag="warm")
            nc.vector.memset(warm[:], 0.0)
            nc.scalar.activation(warm[:], warm[:], EXP)

            # ---- prologue: load, (split,) transpose (chunked) ----
            def load_transposed(src_dt, nat_f32, dstT, q, mtag):
                ts = slice(q * QT, (q + 1) * QT)
                nc.gpsimd.dma_start(nat_f32[:, ts], src_dt[:, ts])
                nat2 = nat_f32[:, ts].rearrange("p a b -> p (a b)")
                hi = prolog.tile([P, QW], F16, tag=f"hi_{mtag}")
                nc.vector.tensor_copy(hi[:], nat2)
                if precise:
                    lo = prolog.tile([P, QW], F16, tag=f"lo_{mtag}")
                    nc.vector.tensor_sub(lo[:], nat2, hi[:])
                    hiT = prolog.tile([P, QT, P], F16, tag=f"hiT_{mtag}")
                    loT = prolog.tile([P, QT, P], F16, tag=f"loT_{mtag}")
                    nc.sync.dma_start_transpose(hiT[:], hi[:])
                    nc.sync.dma_start_transpose(loT[:], lo[:])
                    nc.vector.tensor_add(dstT[:, ts], hiT[:], loT[:])
                else:
                    nc.sync.dma_start_transpose(dstT[:, ts], hi[:])

            # main first: mm1 needs all of mainT but only one c-tile of ctxT
            for q in range(NQ):
                load_transposed(main_dt, main_f32, mainT, q, "m")
            for q in range(NQ):
                load_transposed(ctx_dt, ctx_f32, ctxT, q, "c")

            # ---- main loop over c-tiles ----
            acc = psum_acc.tile([P, S], F32)  # weightedT accumulator [d, m]
            for ct in range(T):
                e_t = etile_pool.tile([P, S], EDT, tag="e")
                s_part = small.tile([P, 2], F32, tag="spart")
                for h in range(2):
                    w_ps = psum_w.tile([P, HALF], F32, tag="w")
                    for j in range(0, HALF, 512):
                        w = min(512, HALF - j)
                        nc.tensor.matmul(
                            w_ps[:, j:j + w],
                            ctxT[:, ct],
                            mainT2[:, h * HALF + j: h * HALF + j + w],
                            start=True,
                            stop=True,
                        )
                    nc.scalar.activation(
                        e_t[:, h * HALF:(h + 1) * HALF],
                        w_ps[:],
                        EXP,
                        accum_out=s_part[:, h:h + 1],
                    )
                s_sum = small.tile([P, 1], F32, tag="ssum")
                nc.vector.tensor_add(s_sum[:], s_part[:, 0:1], s_part[:, 1:2])
                sinv = small.tile([P, 1], F32, tag="sinv")
                nc.vector.reciprocal(sinv[:], s_sum[:])
                ctx_s = small.tile([P, P], EDT, tag="ctxs")
                nc.vector.tensor_scalar_mul(ctx_s[:], ctx_f32[:, ct], sinv[:])
                for j in range(0, S, 512):
                    w = min(512, S - j)
                    nc.tensor.matmul(
                        acc[:, j:j + w],
                        ctx_s[:],
                        e_t[:, j:j + w],
                        start=(ct == 0),
                        stop=(ct == T - 1),
                    )

            # ---- tail: evac, transpose back, subtract, store ----
            for q in range(NQ):
                ts = slice(q * QT, (q + 1) * QT)
                cs = slice(q * QW, (q + 1) * QW)
                w_hi = tailp.tile([P, QW], F16, tag="whi")
                nc.vector.tensor_copy(w_hi[:], acc[:, cs])
                hi_nat = tailp.tile([P, QT, P], F16, tag="hinat")
                nc.sync.dma_start_transpose(hi_nat[:], w_hi[:])
                out_sb = tailp.tile([P, QT, P], F32, tag="outsb")
                nc.vector.tensor_sub(out_sb[:], main_f32[:, ts], hi_nat[:])
                if TAIL_HILO:
                    w_lo = tailp.tile([P, QW], F16, tag="wlo")
                    nc.vector.tensor_sub(w_lo[:], acc[:, cs], w_hi[:])
                    lo_nat = tailp.tile([P, QT, P], F16, tag="lonat")
                    nc.sync.dma_start_transpose(lo_nat[:], w_lo[:])
                    nc.vector.tensor_sub(out_sb[:], out_sb[:], lo_nat[:])
                nc.scalar.dma_start(out_dt[:, ts], out_sb[:])

    nc.compile()
    return nc


_NC_CACHE = {}


def _get_nc(S, D):
    key = (S, D)
    if key not in _NC_CACHE:
        _NC_CACHE[key] = build_nc(S, D)
    return _NC_CACHE[key]


def kernel(context: np.ndarray, main: np.ndarray) -> np.ndarray:
    B, S, D = context.shape
    assert main.shape == (B, S, D) and B == N_CORES
    nc = _get_nc(S, D)
    in_maps = [
        {
            "context": np.ascontiguousarray(context[b], dtype=np.float32),
            "main": np.ascontiguousarray(main[b], dtype=np.float32),
        }
        for b in range(B)
    ]
    res = run_bass_kernel_spmd(nc, in_maps, list(range(N_CORES)))
    return np.stack([res.results[b]["out"] for b in range(B)], axis=0)


# revision 7
# speedup vs baseline: 1.0290x; 1.0290x over previous
"""Trainium2 Bass kernel for AlignOnlySubLayer.

Per batch b:
    W[c,m]   = sum_d context[b,c,d] * main[b,m,d]
    A        = softmax(W, axis=m)
    out[m,d] = main[b,m,d] - sum_c A[c,m] * context[b,c,d]

Sharding: data-parallel over batch B=8 across the 8 NeuronCores (one batch
per core, no cross-core communication).

Kernel design (per core), driven by trn2 hardware facts:
  - PE runs fp32 matmuls at 1/4 rate but fp16/bf16 at 1 row/cycle, so both
    matmuls run in 16-bit with f32 PSUM accumulation. mm1 uses fp16 (4x the
    mantissa of bf16). mm2's operands must be bf16: E = exp(W) reaches
    ~e^70 (no max-subtraction), far beyond fp16 range; bf16 keeps the f32
    exponent. Measured error: ~2e-3 relative l2, ~3e-3 scale-relative absmax
    (c.f. ~1.4e-2 scale-relative for a plain bf16 kernel).
  - Skipping the row-max subtraction is safe (|W| <= ~70 << 88.7 f32 exp
    overflow) and exact (softmax is shift-invariant); it avoids a full
    reduce pass over the 2048x2048 scores.
  - Both matmuls need the contraction dim on partitions, so context/main
    are loaded as fp16 (SWDGE dtype-cast DMA) and transposed d-major with
    the DMA xbar transpose (2-byte dtypes only), quarter-chunked and
    interleaved so mm1 starts after only 2 of 8 transposes.
  - ACT Exp reads W straight from PSUM, writes bf16 E, and its fused
    accum_out yields one half's row-sum; the other half's row-sum is
    reduced on DVE to keep ACT (the critical engine: 4M exps at
    1 elem/lane/cycle) lean.
  - Softmax normalization is folded into context (ctx_s = context / S[c]):
    scales a 2048x128 matrix instead of the 2048x2048 weights.
  - mm2 accumulates weightedT[d,m] over all 16 c-chunks in a persistent
    4-bank PSUM region (the other 4 banks double-buffer mm1).
  - Tail: weightedT -> fp16, xbar-transpose back to [m,d], subtract from
    fp16 main into f32 out, streamed per quarter.
"""

import numpy as np

import concourse.bass as bass
import concourse.mybir as mybir
from concourse import bacc
from concourse.tile import TileContext
from concourse.bass_utils import run_bass_kernel_spmd

P = 128
F32 = mybir.dt.float32
F16 = mybir.dt.float16
BF16 = mybir.dt.bfloat16
EXP = mybir.ActivationFunctionType.Exp
AX = mybir.AxisListType.X
ADD = mybir.AluOpType.add
N_CORES = 8


def build_nc(S=2048, D=128, num_devices=N_CORES, repeats=1, precise=False):
    """Build the single-core Bass program (SPMD across cores)."""
    assert D == P and S % P == 0
    T = S // P            # number of 128-row tiles along c (and m)
    QT = max(1, T // 4)   # tiles per prologue/tail chunk
    NQ = T // QT          # number of chunks
    QW = QT * P           # columns per chunk
    HALF = S // 2         # columns per mm1 psum half

    nc = bacc.Bacc(
        "TRN2",
        target_bir_lowering=False,
        debug=False,
        enable_asserts=False,
        num_devices=num_devices,
    )
    ctx_d = nc.dram_tensor("context", [S, D], F32, kind="ExternalInput").ap()
    main_d = nc.dram_tensor("main", [S, D], F32, kind="ExternalInput").ap()
    out_d = nc.dram_tensor("out", [S, D], F32, kind="ExternalOutput").ap()

    ctx_dt = ctx_d.rearrange("(t p) d -> p t d", p=P)
    main_dt = main_d.rearrange("(t p) d -> p t d", p=P)
    out_dt = out_d.rearrange("(t p) d -> p t d", p=P)

    with TileContext(nc) as tc:
      for _rep in range(repeats):
        with (
            tc.tile_pool(name="persist", bufs=1) as persist,
            tc.tile_pool(name="etile", bufs=3) as etile_pool,
            tc.tile_pool(name="small", bufs=3) as small,
            tc.tile_pool(name="tailp", bufs=2) as tailp,
            tc.tile_pool(name="psum_w", bufs=2, space="PSUM") as psum_w,
            tc.tile_pool(name="psum_acc", bufs=1, space="PSUM") as psum_acc,
        ):
            # ---- persistent SBUF tensors ----
            ctx_h = persist.tile([P, T, P], F16)     # natural [c_in, ct, d]
            main_h = persist.tile([P, T, P], F16)    # natural [m_in, mt, d]
            ctxT = persist.tile([P, T, P], F16)      # [d, ct, c_in]
            mainT = persist.tile([P, T, P], F16)     # [d, mt, m_in]
            mainT2 = mainT.rearrange("p a b -> p (a b)")

            # Warm the ACT exp table early so the ~2.7us table load overlaps
            # the prologue DMAs.
            warm = small.tile([P, 1], F32, tag="warm")
            nc.vector.memset(warm[:], 0.0)
            nc.scalar.activation(warm[:], warm[:], EXP)

            # ---- prologue: fp16 cast-loads, then xbar transposes ----
            # Interleave main/ctx so mm1(ct=0) can start after the first
            # main quarter + first ctx quarter are transposed.
            order = [("m", 0), ("c", 0), ("m", 1), ("m", 2), ("m", 3),
                     ("c", 1), ("c", 2), ("c", 3)]
            order = [(w, q) for (w, q) in order if q < NQ]
            for w, q in order:
                src, nat = (main_dt, main_h) if w == "m" else (ctx_dt, ctx_h)
                ts = slice(q * QT, (q + 1) * QT)
                nc.gpsimd.dma_start(nat[:, ts], src[:, ts])  # SWDGE f32->fp16
            for w, q in order:
                nat, dstT = (main_h, mainT) if w == "m" else (ctx_h, ctxT)
                ts = slice(q * QT, (q + 1) * QT)
                nc.sync.dma_start_transpose(
                    dstT[:, ts], nat[:, ts].rearrange("p a b -> p (a b)")
                )

            # ---- main loop over c-tiles ----
            acc = psum_acc.tile([P, S], F32)  # weightedT accumulator [d, m]
            for ct in range(T):
                e_t = etile_pool.tile([P, S], BF16, tag="e")
                s_part = small.tile([P, 2], F32, tag="spart")
                for h in range(2):
                    w_ps = psum_w.tile([P, HALF], F32, tag="w")
                    for j in range(0, HALF, 512):
                        w = min(512, HALF - j)
                        nc.tensor.matmul(
                            w_ps[:, j:j + w],
                            ctxT[:, ct],
                            mainT2[:, h * HALF + j: h * HALF + j + w],
                            start=True,
                            stop=True,
                        )
                    if h == 0:
                        # row-sum of this half on DVE (off the ACT critical
                        # path); runs concurrently with ACT's exp read.
                        nc.scalar.activation(
                            e_t[:, 0:HALF], w_ps[:], EXP,
                        )
                        nc.vector.tensor_reduce(
                            s_part[:, 0:1], e_t[:, 0:HALF], axis=AX, op=ADD,
                        )
                    else:
                        nc.scalar.activation(
                            e_t[:, HALF:S], w_ps[:], EXP,
                            accum_out=s_part[:, 1:2],
                        )
                s_sum = small.tile([P, 1], F32, tag="ssum")
                nc.vector.tensor_add(s_sum[:], s_part[:, 0:1], s_part[:, 1:2])
                sinv = small.tile([P, 1], F32, tag="sinv")
                nc.vector.reciprocal(sinv[:], s_sum[:])
                ctx_s = small.tile([P, P], BF16, tag="ctxs")
                nc.vector.tensor_scalar_mul(ctx_s[:], ctx_h[:, ct], sinv[:])
                for j in range(0, S, 512):
                    w = min(512, S - j)
                    nc.tensor.matmul(
                        acc[:, j:j + w],
                        ctx_s[:],
                        e_t[:, j:j + w],
                        start=(ct == 0),
                        stop=(ct == T - 1),
                    )

            # ---- tail: evac, transpose back, subtract, store ----
            for q in range(NQ):
                ts = slice(q * QT, (q + 1) * QT)
                cs = slice(q * QW, (q + 1) * QW)
                w_hi = tailp.tile([P, QW], F16, tag="whi")
                nc.vector.tensor_copy(w_hi[:], acc[:, cs])
                hi_nat = tailp.tile([P, QT, P], F16, tag="hinat")
                nc.sync.dma_start_transpose(hi_nat[:], w_hi[:])
                out_sb = tailp.tile([P, QT, P], F32, tag="outsb")
                nc.vector.tensor_sub(out_sb[:], main_h[:, ts], hi_nat[:])
                nc.scalar.dma_start(out_dt[:, ts], out_sb[:])

    nc.compile()
    return nc


_NC_CACHE = {}


def _get_nc(S, D):
    key = (S, D)
    if key not in _NC_CACHE:
        _NC_CACHE[key] = build_nc(S, D)
    return _NC_CACHE[key]


def kernel(context: np.ndarray, main: np.ndarray) -> np.ndarray:
    B, S, D = context.shape
    assert main.shape == (B, S, D) and B == N_CORES
    nc = _get_nc(S, D)
    in_maps = [
        {
            "context": np.ascontiguousarray(context[b], dtype=np.float32),
            "main": np.ascontiguousarray(main[b], dtype=np.float32),
        }
        for b in range(B)
    ]
    res = run_bass_kernel_spmd(nc, in_maps, list(range(N_CORES)))
    return np.stack([res.results[b]["out"] for b in range(B)], axis=0)


# revision 8
# speedup vs baseline: 1.2211x; 1.1867x over previous
"""Trainium2 Bass kernel for AlignOnlySubLayer.

Per batch b:
    W[c,m]   = sum_d context[b,c,d] * main[b,m,d]
    A        = softmax(W, axis=m)
    out[m,d] = main[b,m,d] - sum_c A[c,m] * context[b,c,d]

Sharding: data-parallel over batch B=8 across the 8 NeuronCores (one batch
per core, no cross-core communication).

Kernel design (per core), driven by trn2 hardware facts:
  - PE runs fp32 matmuls at 1/4 rate but fp16/bf16 at 1 row/cycle, so both
    matmuls run in 16-bit with f32 PSUM accumulation. mm1 uses fp16 (4x the
    mantissa of bf16). mm2's operands must be bf16: E = exp(W) reaches
    ~e^70 (no max-subtraction), far beyond fp16 range; bf16 keeps the f32
    exponent. Measured error: ~1.8e-3 relative l2, ~3e-3 scale-relative
    absmax (c.f. ~1.4e-2 scale-relative for a plain bf16 kernel).
  - Skipping the row-max subtraction is safe (|W| <= ~70 << 88.7 f32 exp
    overflow) and exact (softmax is shift-invariant); it avoids a full
    reduce pass over the 2048x2048 scores.
  - Both matmuls need the contraction dim on partitions, so context/main
    are loaded as fp16 (SWDGE dtype-cast DMA) and transposed d-major on the
    TensorEngine (128x128 transpose-mode matmuls into PSUM, evacuated by
    DVE). The DMA xbar transpose is avoided entirely: Tile serializes it
    against other DMA traffic, which was measured to cost ~25us of
    prologue/tail serialization.
  - ACT Exp reads W straight from PSUM, writes bf16 E; one half's row-sum
    comes from ACT's fused accum_out, the other is reduced on DVE, keeping
    ACT (the critical engine: 4M exps at 1 elem/lane/cycle) lean.
  - Softmax normalization is folded into context (ctx_s = context / S[c]):
    scales a 2048x128 matrix instead of the 2048x2048 weights.
  - mm2 accumulates weightedT[d,m] over all 16 c-chunks in a persistent
    4-bank PSUM region (the other 4 banks double-buffer mm1's scores).
    mm2 for tile ct is emitted after mm1 for tile ct+1, so the softmax
    statistics chain (exp -> sums -> reciprocal -> scale) of tile ct
    overlaps the matmuls of tile ct+1 instead of stalling the PE.
  - Tail: weightedT -> fp16, PE-transpose back to [m,d], subtract from
    fp16 main into f32 out, streamed per quarter.
"""

import numpy as np

import concourse.bass as bass
import concourse.mybir as mybir
from concourse import bacc
from concourse.masks import make_identity
from concourse.tile import TileContext
from concourse.bass_utils import run_bass_kernel_spmd

P = 128
F32 = mybir.dt.float32
F16 = mybir.dt.float16
BF16 = mybir.dt.bfloat16
EXP = mybir.ActivationFunctionType.Exp
AX = mybir.AxisListType.X
ADD = mybir.AluOpType.add
N_CORES = 8


def build_nc(S=2048, D=128, num_devices=N_CORES, repeats=1, precise=False):
    """Build the single-core Bass program (SPMD across cores)."""
    assert D == P and S % P == 0
    T = S // P            # number of 128-row tiles along c (and m)
    QT = max(1, T // 4)   # tiles per prologue/tail chunk
    NQ = T // QT          # number of chunks
    QW = QT * P           # columns per chunk
    HALF = S // 2         # columns per mm1 psum half

    nc = bacc.Bacc(
        "TRN2",
        target_bir_lowering=False,
        debug=False,
        enable_asserts=False,
        num_devices=num_devices,
    )
    ctx_d = nc.dram_tensor("context", [S, D], F32, kind="ExternalInput").ap()
    main_d = nc.dram_tensor("main", [S, D], F32, kind="ExternalInput").ap()
    out_d = nc.dram_tensor("out", [S, D], F32, kind="ExternalOutput").ap()

    ctx_dt = ctx_d.rearrange("(t p) d -> p t d", p=P)
    main_dt = main_d.rearrange("(t p) d -> p t d", p=P)
    out_dt = out_d.rearrange("(t p) d -> p t d", p=P)

    with TileContext(nc) as tc:
      for _rep in range(repeats):
        with (
            tc.tile_pool(name="persist", bufs=1) as persist,
            tc.tile_pool(name="etile", bufs=3) as etile_pool,
            tc.tile_pool(name="small", bufs=3) as small,
            tc.tile_pool(name="tailp", bufs=2) as tailp,
            tc.tile_pool(name="psum_w", bufs=2, space="PSUM") as psum_w,
            tc.tile_pool(name="psum_acc", bufs=1, space="PSUM") as psum_acc,
        ):
            # ---- persistent SBUF tensors ----
            ctx_h = persist.tile([P, T, P], F16)     # natural [c_in, ct, d]
            main_h = persist.tile([P, T, P], F16)    # natural [m_in, mt, d]
            ctxT = persist.tile([P, T, P], F16)      # [d, ct, c_in]
            mainT = persist.tile([P, T, P], F16)     # [d, mt, m_in]
            mainT2 = mainT.rearrange("p a b -> p (a b)")
            ident = persist.tile([P, P], F16)
            make_identity(nc, ident[:])

            # Warm the ACT exp table early so the ~2.7us table load overlaps
            # the prologue DMAs.
            warm = small.tile([P, 1], F32, tag="warm")
            nc.vector.memset(warm[:], 0.0)
            nc.scalar.activation(warm[:], warm[:], EXP)

            def pe_transpose_chunk(nat, dstT, ts):
                """Transpose QT natural 128x128 fp16 tiles into dstT[:, ts]
                via PE transpose-mode matmuls, staged through a psum_w slot
                (viewed as fp16), then one DVE evacuation."""
                tw = psum_w.tile([P, HALF], F32, tag="w")
                tw16 = tw.bitcast(F16)
                nt = ts.stop - ts.start
                for t in range(nt):
                    nc.tensor.transpose(
                        tw16[:, t * P:(t + 1) * P],
                        nat[:, ts.start + t],
                        ident[:],
                    )
                nc.vector.tensor_copy(
                    dstT[:, ts].rearrange("p a b -> p (a b)"),
                    tw16[:, 0:nt * P],
                )

            # ---- prologue: fp16 cast-loads, then PE transposes ----
            order = [("m", 0), ("c", 0), ("m", 1), ("m", 2), ("m", 3),
                     ("c", 1), ("c", 2), ("c", 3)]
            order = [(w, q) for (w, q) in order if q < NQ]
            for w, q in order:
                src, nat = (main_dt, main_h) if w == "m" else (ctx_dt, ctx_h)
                ts = slice(q * QT, (q + 1) * QT)
                nc.gpsimd.dma_start(nat[:, ts], src[:, ts])  # SWDGE f32->fp16
            for w, q in order:
                nat, dstT = (main_h, mainT) if w == "m" else (ctx_h, ctxT)
                pe_transpose_chunk(nat, dstT, slice(q * QT, (q + 1) * QT))

            # ---- main loop over c-tiles (mm2 deferred by one tile) ----
            acc = psum_acc.tile([P, S], F32)  # weightedT accumulator [d, m]
            prev = None

            def emit_mm2(ct, e_t, ctx_s):
                for j in range(0, S, 512):
                    w = min(512, S - j)
                    nc.tensor.matmul(
                        acc[:, j:j + w],
                        ctx_s[:],
                        e_t[:, j:j + w],
                        start=(ct == 0),
                        stop=(ct == T - 1),
                    )

            for ct in range(T):
                e_t = etile_pool.tile([P, S], BF16, tag="e")
                s_part = small.tile([P, 2], F32, tag="spart")
                for h in range(2):
                    w_ps = psum_w.tile([P, HALF], F32, tag="w")
                    for j in range(0, HALF, 512):
                        w = min(512, HALF - j)
                        nc.tensor.matmul(
                            w_ps[:, j:j + w],
                            ctxT[:, ct],
                            mainT2[:, h * HALF + j: h * HALF + j + w],
                            start=True,
                            stop=True,
                        )
                    if h == 0:
                        nc.scalar.activation(e_t[:, 0:HALF], w_ps[:], EXP)
                        # this half's row-sum on DVE, off the ACT critical path
                        nc.vector.tensor_reduce(
                            s_part[:, 0:1], e_t[:, 0:HALF], axis=AX, op=ADD,
                        )
                    else:
                        nc.scalar.activation(
                            e_t[:, HALF:S], w_ps[:], EXP,
                            accum_out=s_part[:, 1:2],
                        )
                if prev is not None:
                    emit_mm2(*prev)
                s_sum = small.tile([P, 1], F32, tag="ssum")
                nc.vector.tensor_add(s_sum[:], s_part[:, 0:1], s_part[:, 1:2])
                sinv = small.tile([P, 1], F32, tag="sinv")
                nc.vector.reciprocal(sinv[:], s_sum[:])
                ctx_s = small.tile([P, P], BF16, tag="ctxs")
                nc.vector.tensor_scalar_mul(ctx_s[:], ctx_h[:, ct], sinv[:])
                prev = (ct, e_t, ctx_s)
            emit_mm2(*prev)

            # ---- tail: evac, PE-transpose back, subtract, store ----
            for q in range(NQ):
                ts = slice(q * QT, (q + 1) * QT)
                cs = slice(q * QW, (q + 1) * QW)
                w_hi = tailp.tile([P, QW], F16, tag="whi")
                nc.vector.tensor_copy(w_hi[:], acc[:, cs])
                tw = psum_w.tile([P, HALF], F32, tag="w")
                tw16 = tw.bitcast(F16)
                for t in range(QT):
                    nc.tensor.transpose(
                        tw16[:, t * P:(t + 1) * P],
                        w_hi[:, t * P:(t + 1) * P],
                        ident[:],
                    )
                out_sb = tailp.tile([P, QT, P], F32, tag="outsb")
                nc.vector.tensor_sub(
                    out_sb.rearrange("p a b -> p (a b)"),
                    main_h[:, ts].rearrange("p a b -> p (a b)"),
                    tw16[:, 0:QW],
                )
                nc.scalar.dma_start(out_dt[:, ts], out_sb[:])

    nc.compile()
    return nc


_NC_CACHE = {}


def _get_nc(S, D):
    key = (S, D)
    if key not in _NC_CACHE:
        _NC_CACHE[key] = build_nc(S, D)
    return _NC_CACHE[key]


def kernel(context: np.ndarray, main: np.ndarray) -> np.ndarray:
    B, S, D = context.shape
    assert main.shape == (B, S, D) and B == N_CORES
    nc = _get_nc(S, D)
    in_maps = [
        {
            "context": np.ascontiguousarray(context[b], dtype=np.float32),
            "main": np.ascontiguousarray(main[b], dtype=np.float32),
        }
        for b in range(B)
    ]
    res = run_bass_kernel_spmd(nc, in_maps, list(range(N_CORES)))
    return np.stack([res.results[b]["out"] for b in range(B)], axis=0)


# revision 10
# speedup vs baseline: 1.4618x; 1.1971x over previous
"""Trainium2 Bass kernel for AlignOnlySubLayer.

Per batch b:
    W[c,m]   = sum_d context[b,c,d] * main[b,m,d]
    A        = softmax(W, axis=m)
    out[m,d] = main[b,m,d] - sum_c A[c,m] * context[b,c,d]

Sharding: data-parallel over batch B=8 across the 8 NeuronCores (one batch
per core, no cross-core communication).

Kernel design (per core), driven by trn2 hardware facts:
  - PE runs fp32 matmuls at 1/4 rate but fp16/bf16 at 1 row/cycle, so both
    matmuls run in 16-bit with f32 PSUM accumulation. mm1 uses fp16 (4x the
    mantissa of bf16). mm2's operands must be bf16: E = exp(W) reaches
    ~e^70 (no max-subtraction), far beyond fp16 range; bf16 keeps the f32
    exponent. Measured error: ~1.8e-3 relative l2, ~3e-3 scale-relative
    absmax (c.f. ~1.4e-2 scale-relative for a plain bf16 kernel).
  - Skipping the row-max subtraction is safe (|W| <= ~70 << 88.7 f32 exp
    overflow) and exact (softmax is shift-invariant); it avoids a full
    reduce pass over the 2048x2048 scores.
  - Both matmuls need the contraction dim on partitions, so context/main
    are loaded as fp16 (SWDGE dtype-cast DMA) and transposed d-major on the
    TensorEngine (128x128 transpose-mode matmuls into PSUM, evacuated by
    DVE). The DMA xbar transpose is avoided entirely: Tile serializes it
    against other DMA traffic, which was measured to cost ~25us of
    prologue/tail serialization.
  - ACT Exp reads W straight from PSUM, writes bf16 E; one half's row-sum
    comes from ACT's fused accum_out, the other is reduced on DVE, keeping
    ACT (the critical engine: 4M exps at 1 elem/lane/cycle) lean.
  - Softmax normalization is folded into context (ctx_s = context / S[c]):
    scales a 2048x128 matrix instead of the 2048x2048 weights.
  - mm2 accumulates weightedT[d,m] over all 16 c-chunks in a persistent
    4-bank PSUM region (the other 4 banks double-buffer mm1's scores).
    mm2 for tile ct is emitted after mm1 for tile ct+1, so the softmax
    statistics chain (exp -> sums -> reciprocal -> scale) of tile ct
    overlaps the matmuls of tile ct+1 instead of stalling the PE.
  - Tail: weightedT -> fp16, PE-transpose back to [m,d], subtract from
    fp16 main into f32 out, streamed per quarter.
"""

import numpy as np

import concourse.bass as bass
import concourse.mybir as mybir
from concourse import bacc
from concourse.masks import make_identity
from concourse.tile import TileContext
from concourse.bass_utils import run_bass_kernel_spmd

P = 128
F32 = mybir.dt.float32
F16 = mybir.dt.float16
BF16 = mybir.dt.bfloat16
EXP = mybir.ActivationFunctionType.Exp
AX = mybir.AxisListType.X
ADD = mybir.AluOpType.add
N_CORES = 8


def build_nc(S=2048, D=128, num_devices=N_CORES, repeats=1, precise=False):
    """Build the single-core Bass program (SPMD across cores)."""
    assert D == P and S % P == 0
    T = S // P            # number of 128-row tiles along c (and m)
    QT = max(1, T // 4)   # tiles per prologue/tail chunk
    NQ = T // QT          # number of chunks
    QW = QT * P           # columns per chunk
    HALF = S // 2         # columns per mm1 psum half

    nc = bacc.Bacc(
        "TRN2",
        target_bir_lowering=False,
        debug=False,
        enable_asserts=False,
        num_devices=num_devices,
    )
    ctx_d = nc.dram_tensor("context", [S, D], F32, kind="ExternalInput").ap()
    main_d = nc.dram_tensor("main", [S, D], F32, kind="ExternalInput").ap()
    out_d = nc.dram_tensor("out", [S, D], F32, kind="ExternalOutput").ap()

    ctx_dt = ctx_d.rearrange("(t p) d -> p t d", p=P)
    main_dt = main_d.rearrange("(t p) d -> p t d", p=P)
    out_dt = out_d.rearrange("(t p) d -> p t d", p=P)

    with TileContext(nc) as tc:
      for _rep in range(repeats):
        with (
            tc.tile_pool(name="persist", bufs=1) as persist,
            tc.tile_pool(name="etile", bufs=3) as etile_pool,
            tc.tile_pool(name="small", bufs=3) as small,
            tc.tile_pool(name="tailp", bufs=2) as tailp,
            tc.tile_pool(name="psum_w", bufs=2, space="PSUM") as psum_w,
            tc.tile_pool(name="psum_acc", bufs=1, space="PSUM") as psum_acc,
        ):
            # ---- persistent SBUF tensors ----
            ctx_h = persist.tile([P, T, P], F16)     # natural [c_in, ct, d]
            main_h = persist.tile([P, T, P], F16)    # natural [m_in, mt, d]
            ctxT = persist.tile([P, T, P], F16)      # [d, ct, c_in]
            mainT = persist.tile([P, T, P], F16)     # [d, mt, m_in]
            mainT2 = mainT.rearrange("p a b -> p (a b)")
            ident = persist.tile([P, P], F16)
            make_identity(nc, ident[:])

            # Warm the ACT exp table early so the ~2.7us table load overlaps
            # the prologue DMAs.
            warm = small.tile([P, 1], F32, tag="warm")
            nc.vector.memset(warm[:], 0.0)
            nc.scalar.activation(warm[:], warm[:], EXP)

            def pe_transpose_chunk(nat, dstT, ts):
                """Transpose QT natural 128x128 fp16 tiles into dstT[:, ts]
                via PE transpose-mode matmuls, staged through a psum_w slot
                (viewed as fp16), then one DVE evacuation."""
                tw = psum_w.tile([P, HALF], F32, tag="w")
                tw16 = tw.bitcast(F16)
                nt = ts.stop - ts.start
                for t in range(nt):
                    nc.tensor.transpose(
                        tw16[:, t * P:(t + 1) * P],
                        nat[:, ts.start + t],
                        ident[:],
                    )
                nc.vector.tensor_copy(
                    dstT[:, ts].rearrange("p a b -> p (a b)"),
                    tw16[:, 0:nt * P],
                )

            # ---- prologue: fp16 cast-loads, then PE transposes ----
            order = [("m", 0), ("c", 0), ("m", 1), ("m", 2), ("m", 3),
                     ("c", 1), ("c", 2), ("c", 3)]
            order = [(w, q) for (w, q) in order if q < NQ]
            for w, q in order:
                srcd, nat = (main_dt, main_h) if w == "m" else (ctx_dt, ctx_h)
                ts = slice(q * QT, (q + 1) * QT)
                raw = tailp.tile([P, QT, P], F32, tag=f"ld_{w}")
                nc.sync.dma_start(raw[:], srcd[:, ts])
                nc.vector.tensor_copy(
                    nat[:, ts].rearrange("p a b -> p (a b)"),
                    raw.rearrange("p a b -> p (a b)"),
                )
            for w, q in order:
                nat, dstT = (main_h, mainT) if w == "m" else (ctx_h, ctxT)
                pe_transpose_chunk(nat, dstT, slice(q * QT, (q + 1) * QT))

            # ---- main loop over c-tiles (mm2 deferred by one tile) ----
            # acc holds weighted in natural [m, d] layout: one [128, 128]
            # accumulator per m-block, so no transpose is needed at the end.
            acc = psum_acc.tile([P, T, P], F32)
            prev = None

            # PSUM start=True marks the whole 2KB zero-region (bank) as
            # pending-zero, so only the first sub-block of each bank issues
            # it; the other blocks' first writes land on pending-zero bytes
            # and overwrite, then everything accumulates. skip_group_check
            # silences the sim's region-granular group tracker.
            BPB = 2048 // (P * 4)  # 512B blocks per 2KB bank = 4

            def emit_mm2(ct, e_t, ctx_s):
                for mb in range(T):
                    nc.tensor.matmul(
                        acc[:, mb],
                        e_t[:, mb * P:(mb + 1) * P],
                        ctx_s[:],
                        start=(ct == 0 and mb % BPB == 0),
                        stop=(ct == T - 1),
                        skip_group_check=True,
                    )

            for ct in range(T):
                e_t = etile_pool.tile([P, S], BF16, tag="e")
                s_part = small.tile([P, 2], F32, tag="spart")
                for h in range(2):
                    w_ps = psum_w.tile([P, HALF], F32, tag="w")
                    for j in range(0, HALF, 512):
                        w = min(512, HALF - j)
                        nc.tensor.matmul(
                            w_ps[:, j:j + w],
                            ctxT[:, ct],
                            mainT2[:, h * HALF + j: h * HALF + j + w],
                            start=True,
                            stop=True,
                        )
                    if h == 0:
                        nc.scalar.activation(e_t[:, 0:HALF], w_ps[:], EXP)
                        # this half's row-sum on DVE, off the ACT critical path
                        nc.vector.tensor_reduce(
                            s_part[:, 0:1], e_t[:, 0:HALF], axis=AX, op=ADD,
                        )
                    else:
                        nc.scalar.activation(
                            e_t[:, HALF:S], w_ps[:], EXP,
                            accum_out=s_part[:, 1:2],
                        )
                if prev is not None:
                    emit_mm2(*prev)
                s_sum = small.tile([P, 1], F32, tag="ssum")
                nc.vector.tensor_add(s_sum[:], s_part[:, 0:1], s_part[:, 1:2])
                sinv = small.tile([P, 1], F32, tag="sinv")
                nc.vector.reciprocal(sinv[:], s_sum[:])
                ctx_s = small.tile([P, P], BF16, tag="ctxs")
                nc.vector.tensor_scalar_mul(ctx_s[:], ctx_h[:, ct], sinv[:])
                prev = (ct, e_t, ctx_s)
            emit_mm2(*prev)

            # ---- tail: subtract from PSUM, store ----
            for q in range(NQ):
                ts = slice(q * QT, (q + 1) * QT)
                out_sb = tailp.tile([P, QT, P], F32, tag="outsb")
                nc.vector.tensor_sub(
                    out_sb.rearrange("p a b -> p (a b)"),
                    main_h[:, ts].rearrange("p a b -> p (a b)"),
                    acc[:, ts].rearrange("p a b -> p (a b)"),
                )
                nc.scalar.dma_start(out_dt[:, ts], out_sb[:])

    nc.compile()
    return nc


_NC_CACHE = {}


def _get_nc(S, D):
    key = (S, D)
    if key not in _NC_CACHE:
        _NC_CACHE[key] = build_nc(S, D)
    return _NC_CACHE[key]


def kernel(context: np.ndarray, main: np.ndarray) -> np.ndarray:
    B, S, D = context.shape
    assert main.shape == (B, S, D) and B == N_CORES
    nc = _get_nc(S, D)
    in_maps = [
        {
            "context": np.ascontiguousarray(context[b], dtype=np.float32),
            "main": np.ascontiguousarray(main[b], dtype=np.float32),
        }
        for b in range(B)
    ]
    res = run_bass_kernel_spmd(nc, in_maps, list(range(N_CORES)))
    return np.stack([res.results[b]["out"] for b in range(B)], axis=0)
